# revision 23
# baseline (speedup 1.0000x reference)
"""Causal multi-head attention (B=16, T=1024, E=1024, H=16, Dh=64) on 8 TRN2
NeuronCores.

Sharding: data-parallel over batch -- 2 batch elements per core, weights
replicated, no collectives. Host pre-transposes x and packs weights; each core
runs an identical Bass/Tile program on its shard.

Per-core dataflow (all in "transposed" orientation so no on-chip transposes
are ever needed):
  x^T [E,T] (host)   --matmul-->  Q^T,K^T [Dh,T] per head (head-pairs packed
                                  into 128 partitions; 1/sqrt(Dh) folded into
                                  the Q PSUM->SBUF copy)
                     --matmul-->  V [T,Dh] per head (+ ones column)
  S^T[tk,tq] = (K^T tile).T @ Q^T  per key-tile, causal tiles skipped
  P^T = exp(S^T) on ScalarE (scores are O(1): no max subtraction needed);
        diagonal tiles masked by 0/1 multiply
  O'^T[65,tq] += (V'|1).T @ P^T   -- row 64 accumulates the softmax denom
  Y^T = O'^T[0:64] * bcast(1/denom)
  out[t,E] = Y^T.T @ Wo + bo
"""
import numpy as np
import ml_dtypes

import concourse.bass as bass
import concourse.mybir as mybir
import concourse.tile as tile
from concourse import bacc
from concourse.bass_utils import run_bass_kernel_spmd

B, T, E = 16, 1024, 1024
H, Dh = 16, 64
NCORES = 8
BL = B // NCORES          # batches per core
P = 128                   # partitions
ET = E // P               # 8 tiles along E / token / hd dims
HP = H // 2               # 8 head-pairs
BF = mybir.dt.bfloat16
F32 = mybir.dt.float32
AF = mybir.ActivationFunctionType

_CACHE = {}


def _pieces(i):
    """Column pieces of [128*i, 1024) that do not cross the 512 PSUM-bank
    boundary."""
    if i < 4:
        return [(128 * i, 512), (512, 1024)]
    return [(128 * i, 1024)]


def _build(dbg=False):
    nc = bacc.Bacc("TRN2", target_bir_lowering=False, debug=False,
                   num_devices=NCORES)

    dbg_out = {}
    if dbg:
        for name, shape, dt in [
            ("d_qT", [P, HP, T], BF), ("d_kT", [P, HP, T], BF),
            ("d_v", [P, ET, H, Dh + 1], BF), ("d_pt", [ET, P, T], BF),
            ("d_s0", [P, T], F32), ("d_op", [P, T], F32),
            ("d_r1", [1, T], F32), ("d_rb", [Dh, T], F32),
            ("d_yT", [P, HP, T], BF),
        ]:
            dbg_out[name] = nc.dram_tensor(name, shape, dt,
                                           kind="ExternalOutput").ap()

    xT = nc.dram_tensor("xT", [BL, E, T], BF, kind="ExternalInput").ap()
    wq = nc.dram_tensor("wq", [E, H * Dh], BF, kind="ExternalInput").ap()
    wk = nc.dram_tensor("wk", [E, H * Dh], BF, kind="ExternalInput").ap()
    wv = nc.dram_tensor("wv", [E, H * Dh], BF, kind="ExternalInput").ap()
    wo = nc.dram_tensor("wo", [H * Dh, E], BF, kind="ExternalInput").ap()
    borep = nc.dram_tensor("borep", [P, E], F32, kind="ExternalInput").ap()
    mask01 = nc.dram_tensor("mask01", [P, P], BF, kind="ExternalInput").ap()
    out = nc.dram_tensor("out", [BL, T, E], F32, kind="ExternalOutput").ap()

    with tile.TileContext(nc) as tc:
        with (
            tc.tile_pool(name="consts", bufs=1) as cpool,
            tc.tile_pool(name="xp", bufs=1) as xpool,
            tc.tile_pool(name="qk", bufs=1) as qkpool,
            tc.tile_pool(name="vy", bufs=1) as vypool,
            tc.tile_pool(name="pt", bufs=3) as ptpool,
            tc.tile_pool(name="sm", bufs=2) as spool,
            tc.tile_pool(name="dn", bufs=1) as dnpool,
            tc.tile_pool(name="ob", bufs=3) as opool,
            tc.tile_pool(name="ps", bufs=4, space="PSUM") as ps,
        ):
            wq_sb = cpool.tile([P, ET, H * Dh], BF, tag="wq")
            wk_sb = cpool.tile([P, ET, H * Dh], BF, tag="wk")
            wv_sb = cpool.tile([P, ET, H * Dh], BF, tag="wv")
            wo_sb = cpool.tile([P, ET, E], BF, tag="wo")
            nc.sync.dma_start(wq_sb[:], wq.rearrange("(n p) c -> p n c", p=P))
            nc.sync.dma_start(wk_sb[:], wk.rearrange("(n p) c -> p n c", p=P))
            nc.sync.dma_start(wv_sb[:], wv.rearrange("(n p) c -> p n c", p=P))
            nc.sync.dma_start(wo_sb[:], wo.rearrange("(n p) c -> p n c", p=P))
            borep_sb = cpool.tile([P, E], F32, tag="bo")
            nc.sync.dma_start(borep_sb[:], borep)
            mask_sb = cpool.tile([P, P], BF, tag="mask")
            nc.sync.dma_start(mask_sb[:], mask01)

            for b in range(BL):
                xT_sb = xpool.tile([P, ET, T], BF, tag="xT")
                nc.sync.dma_start(
                    xT_sb[:], xT[b].rearrange("(n p) c -> p n c", p=P))

                # ---- V projection: V'[tok, head, 65] (col 64 = ones) ----
                v_sb = vypool.tile([P, ET, H, Dh + 1], BF, tag="v")
                nc.vector.memset(v_sb[:, :, :, Dh], 1.0)
                for t in range(ET):
                    vp = ps.tile([P, 1024], F32, tag="ps")
                    for n2 in range(2):
                        cs = slice(512 * n2, 512 * (n2 + 1))
                        for i in range(ET):
                            nc.tensor.matmul(
                                vp[:, cs],
                                lhsT=xT_sb[:, i, 128 * t:128 * (t + 1)],
                                rhs=wv_sb[:, i, cs],
                                start=(i == 0), stop=(i == ET - 1),
                            )
                        nc.scalar.activation(
                            v_sb[:, t, 8 * n2:8 * (n2 + 1), 0:Dh],
                            vp[:, cs].rearrange("p (h d) -> p h d", d=Dh),
                            AF.Copy,
                        )

                # ---- Q^T / K^T projections (head-pair packed) ----
                qT = qkpool.tile([P, HP, T], BF, tag="q")
                kT = qkpool.tile([P, HP, T], BF, tag="k")
                for hp in range(HP):
                    qp = ps.tile([P, 1024], F32, tag="ps")
                    kp = ps.tile([P, 1024], F32, tag="ps")
                    for n2 in range(2):
                        cs = slice(512 * n2, 512 * (n2 + 1))
                        for i in range(ET):
                            nc.tensor.matmul(
                                qp[:, cs],
                                lhsT=wq_sb[:, i, 128 * hp:128 * (hp + 1)],
                                rhs=xT_sb[:, i, cs],
                                start=(i == 0), stop=(i == ET - 1),
                            )
                        for i in range(ET):
                            nc.tensor.matmul(
                                kp[:, cs],
                                lhsT=wk_sb[:, i, 128 * hp:128 * (hp + 1)],
                                rhs=xT_sb[:, i, cs],
                                start=(i == 0), stop=(i == ET - 1),
                            )
                    # PSUM->SBUF casts; attention scale folded into Q
                    nc.scalar.activation(qT[:, hp, :], qp[:], AF.Copy,
                                         scale=1.0 / float(np.sqrt(Dh)))
                    nc.scalar.activation(kT[:, hp, :], kp[:], AF.Copy)

                if dbg and b == 0:
                    nc.sync.dma_start(dbg_out["d_qT"], qT[:])
                    nc.sync.dma_start(dbg_out["d_kT"], kT[:])
                    nc.sync.dma_start(dbg_out["d_v"], v_sb[:])

                # ---- attention pass A: all heads, unnormalized ----
                yT = vypool.tile([P, HP, T], BF, tag="y")
                # per-head softmax denominators: engine APs require partition
                # base in {0,32,64,96}, so head h lands at partition
                # 32*(h%4), free slot h//4
                den = dnpool.tile([P, 4, T], F32, tag="den")
                nc.vector.memset(den[:], 1.0)
                for h in range(H):
                    hp, po = h // 2, Dh * (h % 2)
                    op_ = ps.tile([P, 1024], F32, tag="ps")
                    for i in range(ET):
                        sp_ = ps.tile([P, 1024], F32, tag="ps")
                        for (a0, a1) in _pieces(i):
                            nc.tensor.matmul(
                                sp_[:, a0:a1],
                                lhsT=kT[po:po + Dh, hp, 128 * i:128 * (i + 1)],
                                rhs=qT[po:po + Dh, hp, a0:a1],
                                start=True, stop=True,
                            )
                        pt = ptpool.tile([P, 1024], BF, tag="pt")
                        nc.scalar.activation(pt[:, 128 * i:], sp_[:, 128 * i:],
                                             AF.Exp)
                        # zero the below-diagonal half of the diagonal tile
                        ds_ = slice(128 * i, 128 * (i + 1))
                        nc.vector.tensor_mul(pt[:, ds_], pt[:, ds_], mask_sb[:])
                        if dbg and b == 0 and h == 0:
                            if i == 0:
                                s0c = dnpool.tile([P, T], F32, tag="dbg_s0")
                                nc.vector.tensor_copy(s0c[:], sp_[:])
                                nc.sync.dma_start(dbg_out["d_s0"], s0c[:])
                            nc.sync.dma_start(dbg_out["d_pt"][i], pt[:])
                        for (a0, a1) in _pieces(i):
                            nc.tensor.matmul(
                                op_[0:Dh + 1, a0:a1],
                                lhsT=v_sb[:, i, h, :],
                                rhs=pt[:, a0:a1],
                                start=(i == 0), stop=(i == ET - 1),
                                skip_group_check=True,
                            )
                    nc.scalar.activation(yT[po:po + Dh, hp, :], op_[0:Dh, :],
                                         AF.Copy)
                    pb = 32 * (h % 4)
                    nc.vector.tensor_copy(den[pb:pb + 1, h // 4, :],
                                          op_[Dh:Dh + 1, :])
                    if dbg and b == 0 and h == 0:
                        opc = dnpool.tile([P, T], F32, tag="dbg_op")
                        nc.vector.tensor_copy(opc[:], op_[:])
                        nc.sync.dma_start(dbg_out["d_op"], opc[:])

                # ---- pass B: batched exact reciprocal + normalize ----
                nc.vector.reciprocal(den[:], den[:])
                if dbg and b == 0:
                    nc.sync.dma_start(dbg_out["d_r1"], den[0:1, 0, :])
                for h in range(H):
                    hp, po = h // 2, Dh * (h % 2)
                    pb = 32 * (h % 4)
                    # stage to partition 0: partition_broadcast ignores the
                    # AP's partition base on HW (reads garbage for base != 0)
                    r1 = spool.tile([1, T], F32, tag="r1")
                    nc.scalar.activation(r1[:], den[pb:pb + 1, h // 4, :],
                                         AF.Copy)
                    rb = spool.tile([P, T], F32, tag="rb")
                    nc.gpsimd.partition_broadcast(rb[:], r1[:])
                    if dbg and b == 0 and h == 1:
                        nc.sync.dma_start(dbg_out["d_rb"], rb[0:Dh, :])
                    nc.vector.tensor_mul(yT[po:po + Dh, hp, :],
                                         yT[po:po + Dh, hp, :],
                                         rb[po:po + Dh, :])

                if dbg and b == 0:
                    nc.sync.dma_start(dbg_out["d_yT"], yT[:])

                # ---- output projection + bias ----
                for t in range(ET):
                    o2 = ps.tile([P, 1024], F32, tag="ps")
                    for n2 in range(2):
                        cs = slice(512 * n2, 512 * (n2 + 1))
                        for j in range(ET):
                            nc.tensor.matmul(
                                o2[:, cs],
                                lhsT=yT[:, j, 128 * t:128 * (t + 1)],
                                rhs=wo_sb[:, j, cs],
                                start=(j == 0), stop=(j == ET - 1),
                            )
                    ob = opool.tile([P, E], F32, tag="ob")
                    nc.vector.tensor_add(ob[:], o2[:], borep_sb[:])
                    nc.sync.dma_start(out[b, 128 * t:128 * (t + 1), :], ob[:])

    nc.compile()
    return nc


def _get_nc():
    if "nc" not in _CACHE:
        _CACHE["nc"] = _build()
    return _CACHE["nc"]


def _prep_in_maps(x, Wq, Wk, Wv, Wo, bo):
    bf16 = ml_dtypes.bfloat16
    # [B,T,E] -> [B,E,T] transposed activations
    xT = np.ascontiguousarray(x.transpose(0, 2, 1)).astype(bf16)
    # [H,E,Dh] -> [E, H*Dh] (heads side by side so a 128-col slice = 2 heads)
    wq_pk = np.ascontiguousarray(Wq.transpose(1, 0, 2).reshape(E, H * Dh)).astype(bf16)
    wk_pk = np.ascontiguousarray(Wk.transpose(1, 0, 2).reshape(E, H * Dh)).astype(bf16)
    wv_pk = np.ascontiguousarray(Wv.transpose(1, 0, 2).reshape(E, H * Dh)).astype(bf16)
    wo_b = np.ascontiguousarray(Wo).astype(bf16)
    borep = np.ascontiguousarray(
        np.broadcast_to(bo.astype(np.float32), (P, E)))
    ii, jj = np.mgrid[0:P, 0:P]
    mask01 = (jj >= ii).astype(bf16)  # S^T[tk,tq]: keep tq >= tk

    in_maps = []
    for c in range(NCORES):
        in_maps.append({
            "xT": xT[BL * c:BL * (c + 1)],
            "wq": wq_pk, "wk": wk_pk, "wv": wv_pk, "wo": wo_b,
            "borep": borep, "mask01": mask01,
        })
    return in_maps


def run(inputs, trace=False):
    """Returns (full_output [B,T,E] fp32, BassKernelResults)."""
    nc = _get_nc()
    in_maps = _prep_in_maps(**inputs)
    res = run_bass_kernel_spmd(nc, in_maps, core_ids=list(range(NCORES)),
                               trace=trace)
    out = np.concatenate([res.results[c]["out"] for c in range(NCORES)],
                         axis=0)
    return out, res


def kernel(x, Wq, Wk, Wv, Wo, bo):
    out, _ = run(dict(x=x, Wq=Wq, Wk=Wk, Wv=Wv, Wo=Wo, bo=bo))
    return out


# revision 25
# speedup vs baseline: 1.0450x; 1.0450x over previous
"""Causal multi-head attention (B=16, T=1024, E=1024, H=16, Dh=64) on 8 TRN2
NeuronCores.

Sharding: data-parallel over batch -- 2 batch elements per core, weights
replicated, no collectives. Host pre-transposes x and packs weights; each core
runs an identical Bass/Tile program on its shard.

Per-core dataflow (all in "transposed" orientation so no on-chip transposes
are ever needed):
  x^T [E,T] (host)   --matmul-->  Q^T,K^T [Dh,T] per head (head-pairs packed
                                  into 128 partitions; 1/sqrt(Dh) folded into
                                  the Q PSUM->SBUF copy)
                     --matmul-->  V [T,Dh] per head (+ ones column)
  S^T[tk,tq] = (K^T tile).T @ Q^T  per key-tile, causal tiles skipped
  P^T = exp(S^T) on ScalarE (scores are O(1): no max subtraction needed);
        diagonal tiles masked by 0/1 multiply
  O'^T[65,tq] += (V'|1).T @ P^T   -- row 64 accumulates the softmax denom
  Y^T = O'^T[0:64] * bcast(1/denom)
  out[t,E] = Y^T.T @ Wo + bo
"""
import numpy as np
import ml_dtypes

import concourse.bass as bass
import concourse.mybir as mybir
import concourse.tile as tile
from concourse import bacc
from concourse.bass_utils import run_bass_kernel_spmd

B, T, E = 16, 1024, 1024
H, Dh = 16, 64
NCORES = 8
BL = B // NCORES          # batches per core
P = 128                   # partitions
ET = E // P               # 8 tiles along E / token / hd dims
HP = H // 2               # 8 head-pairs
BF = mybir.dt.bfloat16
F32 = mybir.dt.float32
AF = mybir.ActivationFunctionType

_CACHE = {}


def _pieces(i):
    """Column pieces of [128*i, 1024) that do not cross the 512 PSUM-bank
    boundary."""
    if i < 4:
        return [(128 * i, 512), (512, 1024)]
    return [(128 * i, 1024)]


def _build(dbg=False):
    nc = bacc.Bacc("TRN2", target_bir_lowering=False, debug=False,
                   num_devices=NCORES)

    dbg_out = {}
    if dbg:
        for name, shape, dt in [
            ("d_qT", [P, HP, T], BF), ("d_kT", [P, HP, T], BF),
            ("d_v", [P, ET, H, Dh + 1], BF), ("d_pt", [ET, P, T], BF),
            ("d_s0", [P, T], F32), ("d_op", [P, T], F32),
            ("d_r1", [1, T], F32), ("d_rb", [Dh, T], F32),
            ("d_yT", [P, HP, T], BF),
        ]:
            dbg_out[name] = nc.dram_tensor(name, shape, dt,
                                           kind="ExternalOutput").ap()

    xT = nc.dram_tensor("xT", [BL, E, T], BF, kind="ExternalInput").ap()
    wq = nc.dram_tensor("wq", [E, H * Dh], BF, kind="ExternalInput").ap()
    wk = nc.dram_tensor("wk", [E, H * Dh], BF, kind="ExternalInput").ap()
    wv = nc.dram_tensor("wv", [E, H * Dh], BF, kind="ExternalInput").ap()
    wo = nc.dram_tensor("wo", [H * Dh, E], BF, kind="ExternalInput").ap()
    borep = nc.dram_tensor("borep", [P, E], F32, kind="ExternalInput").ap()
    mask01 = nc.dram_tensor("mask01", [P, P], BF, kind="ExternalInput").ap()
    out = nc.dram_tensor("out", [BL, T, E], F32, kind="ExternalOutput").ap()

    with tile.TileContext(nc) as tc:
        with (
            tc.tile_pool(name="consts", bufs=1) as cpool,
            tc.tile_pool(name="xp", bufs=1) as xpool,
            tc.tile_pool(name="qk", bufs=1) as qkpool,
            tc.tile_pool(name="vy", bufs=1) as vypool,
            tc.tile_pool(name="pt", bufs=3) as ptpool,
            tc.tile_pool(name="sm", bufs=2) as spool,
            tc.tile_pool(name="dn", bufs=1) as dnpool,
            tc.tile_pool(name="ob", bufs=3) as opool,
            tc.tile_pool(name="ps", bufs=4, space="PSUM") as ps,
        ):
            wq_sb = cpool.tile([P, ET, H * Dh], BF, tag="wq")
            wk_sb = cpool.tile([P, ET, H * Dh], BF, tag="wk")
            wv_sb = cpool.tile([P, ET, H * Dh], BF, tag="wv")
            wo_sb = cpool.tile([P, ET, E], BF, tag="wo")
            nc.sync.dma_start(wq_sb[:], wq.rearrange("(n p) c -> p n c", p=P))
            nc.sync.dma_start(wk_sb[:], wk.rearrange("(n p) c -> p n c", p=P))
            nc.sync.dma_start(wv_sb[:], wv.rearrange("(n p) c -> p n c", p=P))
            nc.sync.dma_start(wo_sb[:], wo.rearrange("(n p) c -> p n c", p=P))
            borep_sb = cpool.tile([P, E], F32, tag="bo")
            nc.sync.dma_start(borep_sb[:], borep)
            mask_sb = cpool.tile([P, P], BF, tag="mask")
            nc.sync.dma_start(mask_sb[:], mask01)

            for b in range(BL):
                xT_sb = xpool.tile([P, ET, T], BF, tag="xT")
                nc.sync.dma_start(
                    xT_sb[:], xT[b].rearrange("(n p) c -> p n c", p=P))

                # ---- V projection: V'[tok, head, 65] (col 64 = ones) ----
                v_sb = vypool.tile([P, ET, H, Dh + 1], BF, tag="v")
                nc.vector.memset(v_sb[:, :, :, Dh], 1.0)
                for t in range(ET):
                    vp = ps.tile([P, 1024], F32, tag="ps")
                    for n2 in range(2):
                        cs = slice(512 * n2, 512 * (n2 + 1))
                        for i in range(ET):
                            nc.tensor.matmul(
                                vp[:, cs],
                                lhsT=xT_sb[:, i, 128 * t:128 * (t + 1)],
                                rhs=wv_sb[:, i, cs],
                                start=(i == 0), stop=(i == ET - 1),
                            )
                        nc.scalar.activation(
                            v_sb[:, t, 8 * n2:8 * (n2 + 1), 0:Dh],
                            vp[:, cs].rearrange("p (h d) -> p h d", d=Dh),
                            AF.Copy,
                        )

                # ---- Q^T / K^T projections (head-pair packed) ----
                qT = qkpool.tile([P, HP, T], BF, tag="q")
                kT = qkpool.tile([P, HP, T], BF, tag="k")
                for hp in range(HP):
                    qp = ps.tile([P, 1024], F32, tag="ps")
                    kp = ps.tile([P, 1024], F32, tag="ps")
                    for n2 in range(2):
                        cs = slice(512 * n2, 512 * (n2 + 1))
                        for i in range(ET):
                            nc.tensor.matmul(
                                qp[:, cs],
                                lhsT=wq_sb[:, i, 128 * hp:128 * (hp + 1)],
                                rhs=xT_sb[:, i, cs],
                                start=(i == 0), stop=(i == ET - 1),
                            )
                        for i in range(ET):
                            nc.tensor.matmul(
                                kp[:, cs],
                                lhsT=wk_sb[:, i, 128 * hp:128 * (hp + 1)],
                                rhs=xT_sb[:, i, cs],
                                start=(i == 0), stop=(i == ET - 1),
                            )
                    # PSUM->SBUF casts; attention scale folded into Q
                    nc.scalar.activation(qT[:, hp, :], qp[:], AF.Copy,
                                         scale=1.0 / float(np.sqrt(Dh)))
                    nc.scalar.activation(kT[:, hp, :], kp[:], AF.Copy)

                if dbg and b == 0:
                    nc.sync.dma_start(dbg_out["d_qT"], qT[:])
                    nc.sync.dma_start(dbg_out["d_kT"], kT[:])
                    nc.sync.dma_start(dbg_out["d_v"], v_sb[:])

                # ---- attention pass A: all heads, unnormalized ----
                yT = vypool.tile([P, HP, T], BF, tag="y")
                # per-head softmax denominators: engine APs require partition
                # base in {0,32,64,96}, so head h lands at partition
                # 32*(h%4), free slot h//4
                den = dnpool.tile([P, 4, T], F32, tag="den")
                nc.vector.memset(den[:], 1.0)

                def normalize(h):
                    # stage to partition 0 (partition_broadcast ignores the
                    # AP's partition base on HW), replicate, scale
                    hp, po = h // 2, Dh * (h % 2)
                    pb = 32 * (h % 4)
                    r1 = spool.tile([1, T], F32, tag="r1")
                    nc.scalar.activation(r1[:], den[pb:pb + 1, h // 4, :],
                                         AF.Copy)
                    rb = spool.tile([P, T], F32, tag="rb")
                    nc.gpsimd.partition_broadcast(rb[:], r1[:])
                    if dbg and b == 0 and h == 1:
                        nc.sync.dma_start(dbg_out["d_rb"], rb[0:Dh, :])
                    nc.vector.tensor_mul(yT[po:po + Dh, hp, :],
                                         yT[po:po + Dh, hp, :],
                                         rb[po:po + Dh, :])

                for hp in range(HP):
                    # the pair's two heads interleaved so the PE always has
                    # an independent matmul while the other head's exp runs
                    ops = [ps.tile([P, 1024], F32, tag="ps",
                                   name=f"op{b}_{hp}_{s}") for s in range(2)]
                    for i in range(ET):
                        pts = []
                        for sub in (0, 1):
                            po = Dh * sub
                            sp_ = ps.tile([P, 1024], F32, tag="ps")
                            for (a0, a1) in _pieces(i):
                                nc.tensor.matmul(
                                    sp_[:, a0:a1],
                                    lhsT=kT[po:po + Dh, hp,
                                            128 * i:128 * (i + 1)],
                                    rhs=qT[po:po + Dh, hp, a0:a1],
                                    start=True, stop=True,
                                )
                            pt = ptpool.tile([P, 1024], BF, tag="pt")
                            nc.scalar.activation(pt[:, 128 * i:],
                                                 sp_[:, 128 * i:], AF.Exp)
                            ds_ = slice(128 * i, 128 * (i + 1))
                            nc.vector.tensor_mul(pt[:, ds_], pt[:, ds_],
                                                 mask_sb[:])
                            pts.append(pt)
                            if dbg and b == 0 and hp == 0 and sub == 0:
                                if i == 0:
                                    s0c = dnpool.tile([P, T], F32,
                                                      tag="dbg_s0")
                                    nc.vector.tensor_copy(s0c[:], sp_[:])
                                    nc.sync.dma_start(dbg_out["d_s0"],
                                                      s0c[:])
                                nc.sync.dma_start(dbg_out["d_pt"][i], pt[:])
                        for sub in (0, 1):
                            h = 2 * hp + sub
                            for (a0, a1) in _pieces(i):
                                nc.tensor.matmul(
                                    ops[sub][0:Dh + 1, a0:a1],
                                    lhsT=v_sb[:, i, h, :],
                                    rhs=pts[sub][:, a0:a1],
                                    start=(i == 0), stop=(i == ET - 1),
                                    skip_group_check=True,
                                )
                    for sub in (0, 1):
                        h = 2 * hp + sub
                        po = Dh * sub
                        nc.scalar.activation(yT[po:po + Dh, hp, :],
                                             ops[sub][0:Dh, :], AF.Copy)
                        pb = 32 * (h % 4)
                        nc.vector.tensor_copy(den[pb:pb + 1, h // 4, :],
                                              ops[sub][Dh:Dh + 1, :])
                        if dbg and b == 0 and h == 0:
                            opc = dnpool.tile([P, T], F32, tag="dbg_op")
                            nc.vector.tensor_copy(opc[:], ops[sub][:])
                            nc.sync.dma_start(dbg_out["d_op"], opc[:])
                    if hp % 2 == 1:
                        # heads 4g..4g+3 are complete: reciprocal their slot
                        # and normalize them while later heads still compute
                        g = hp // 2
                        nc.vector.reciprocal(den[:, g, :], den[:, g, :])
                        if dbg and b == 0 and g == 0:
                            nc.sync.dma_start(dbg_out["d_r1"],
                                              den[0:1, 0, :])
                        for h in range(4 * g, 4 * g + 4):
                            normalize(h)

                if dbg and b == 0:
                    nc.sync.dma_start(dbg_out["d_yT"], yT[:])

                # ---- output projection + bias ----
                for t in range(ET):
                    o2 = ps.tile([P, 1024], F32, tag="ps")
                    for n2 in range(2):
                        cs = slice(512 * n2, 512 * (n2 + 1))
                        for j in range(ET):
                            nc.tensor.matmul(
                                o2[:, cs],
                                lhsT=yT[:, j, 128 * t:128 * (t + 1)],
                                rhs=wo_sb[:, j, cs],
                                start=(j == 0), stop=(j == ET - 1),
                            )
                    ob = opool.tile([P, E], F32, tag="ob")
                    nc.vector.tensor_add(ob[:], o2[:], borep_sb[:])
                    nc.sync.dma_start(out[b, 128 * t:128 * (t + 1), :], ob[:])

    nc.compile()
    return nc


def _get_nc():
    if "nc" not in _CACHE:
        _CACHE["nc"] = _build()
    return _CACHE["nc"]


def _prep_in_maps(x, Wq, Wk, Wv, Wo, bo):
    bf16 = ml_dtypes.bfloat16
    # [B,T,E] -> [B,E,T] transposed activations
    xT = np.ascontiguousarray(x.transpose(0, 2, 1)).astype(bf16)
    # [H,E,Dh] -> [E, H*Dh] (heads side by side so a 128-col slice = 2 heads)
    wq_pk = np.ascontiguousarray(Wq.transpose(1, 0, 2).reshape(E, H * Dh)).astype(bf16)
    wk_pk = np.ascontiguousarray(Wk.transpose(1, 0, 2).reshape(E, H * Dh)).astype(bf16)
    wv_pk = np.ascontiguousarray(Wv.transpose(1, 0, 2).reshape(E, H * Dh)).astype(bf16)
    wo_b = np.ascontiguousarray(Wo).astype(bf16)
    borep = np.ascontiguousarray(
        np.broadcast_to(bo.astype(np.float32), (P, E)))
    ii, jj = np.mgrid[0:P, 0:P]
    mask01 = (jj >= ii).astype(bf16)  # S^T[tk,tq]: keep tq >= tk

    in_maps = []
    for c in range(NCORES):
        in_maps.append({
            "xT": xT[BL * c:BL * (c + 1)],
            "wq": wq_pk, "wk": wk_pk, "wv": wv_pk, "wo": wo_b,
            "borep": borep, "mask01": mask01,
        })
    return in_maps


def run(inputs, trace=False):
    """Returns (full_output [B,T,E] fp32, BassKernelResults)."""
    nc = _get_nc()
    in_maps = _prep_in_maps(**inputs)
    res = run_bass_kernel_spmd(nc, in_maps, core_ids=list(range(NCORES)),
                               trace=trace)
    out = np.concatenate([res.results[c]["out"] for c in range(NCORES)],
                         axis=0)
    return out, res


def kernel(x, Wq, Wk, Wv, Wo, bo):
    out, _ = run(dict(x=x, Wq=Wq, Wk=Wk, Wv=Wv, Wo=Wo, bo=bo))
    return out


# revision 28
# speedup vs baseline: 1.0664x; 1.0204x over previous
"""Causal multi-head attention (B=16, T=1024, E=1024, H=16, Dh=64) on 8 TRN2
NeuronCores.

Sharding: data-parallel over batch -- 2 batch elements per core, weights
replicated, no collectives. Host pre-transposes x and packs weights; each core
runs an identical Bass/Tile program on its shard.

Per-core dataflow (all in "transposed" orientation so no on-chip transposes
are ever needed):
  x^T [E,T] (host)   --matmul-->  Q^T,K^T [Dh,T] per head (head-pairs packed
                                  into 128 partitions; 1/sqrt(Dh) folded into
                                  the Q PSUM->SBUF copy)
                     --matmul-->  V [T,Dh] per head (+ ones column)
  S^T[tk,tq] = (K^T tile).T @ Q^T  per key-tile, causal tiles skipped
  P^T = exp(S^T) on ScalarE (scores are O(1): no max subtraction needed);
        diagonal tiles masked by 0/1 multiply
  O'^T[65,tq] += (V'|1).T @ P^T   -- row 64 accumulates the softmax denom
  Y^T = O'^T[0:64] * bcast(1/denom)
  out[t,E] = Y^T.T @ Wo + bo
"""
import numpy as np
import ml_dtypes

import concourse.bass as bass
import concourse.mybir as mybir
import concourse.tile as tile
from concourse import bacc
from concourse.bass_utils import run_bass_kernel_spmd

B, T, E = 16, 1024, 1024
H, Dh = 16, 64
NCORES = 8
BL = B // NCORES          # batches per core
P = 128                   # partitions
ET = E // P               # 8 tiles along E / token / hd dims
HP = H // 2               # 8 head-pairs
BF = mybir.dt.bfloat16
F32 = mybir.dt.float32
AF = mybir.ActivationFunctionType

_CACHE = {}


def _pieces(i):
    """Column pieces of [128*i, 1024) that do not cross the 512 PSUM-bank
    boundary."""
    if i < 4:
        return [(128 * i, 512), (512, 1024)]
    return [(128 * i, 1024)]


def _build(dbg=False):
    nc = bacc.Bacc("TRN2", target_bir_lowering=False, debug=False,
                   num_devices=NCORES)

    dbg_out = {}
    if dbg:
        for name, shape, dt in [
            ("d_qT", [P, HP, T], BF), ("d_kT", [P, HP, T], BF),
            ("d_v", [P, ET, H, Dh + 1], BF), ("d_pt", [ET, P, T], BF),
            ("d_s0", [P, T], F32), ("d_op", [P, T], F32),
            ("d_r1", [1, T], F32), ("d_rb", [Dh, T], F32),
            ("d_yT", [P, HP, T], BF),
        ]:
            dbg_out[name] = nc.dram_tensor(name, shape, dt,
                                           kind="ExternalOutput").ap()

    xT = nc.dram_tensor("xT", [BL, E, T], BF, kind="ExternalInput").ap()
    wq = nc.dram_tensor("wq", [E, H * Dh], BF, kind="ExternalInput").ap()
    wk = nc.dram_tensor("wk", [E, H * Dh], BF, kind="ExternalInput").ap()
    wv = nc.dram_tensor("wv", [E, H * Dh], BF, kind="ExternalInput").ap()
    wo = nc.dram_tensor("wo", [H * Dh, E], BF, kind="ExternalInput").ap()
    borep = nc.dram_tensor("borep", [P, E], F32, kind="ExternalInput").ap()
    mask01 = nc.dram_tensor("mask01", [P, P], BF, kind="ExternalInput").ap()
    out = nc.dram_tensor("out", [BL, T, E], F32, kind="ExternalOutput").ap()

    with tile.TileContext(nc) as tc:
        with (
            tc.tile_pool(name="consts", bufs=1) as cpool,
            tc.tile_pool(name="xp", bufs=1) as xpool,
            tc.tile_pool(name="qk", bufs=1) as qkpool,
            tc.tile_pool(name="vy", bufs=1) as vypool,
            tc.tile_pool(name="pt", bufs=4) as ptpool,
            tc.tile_pool(name="sm", bufs=2) as spool,
            tc.tile_pool(name="dn", bufs=1) as dnpool,
            tc.tile_pool(name="ob", bufs=3) as opool,
            tc.tile_pool(name="ps", bufs=4, space="PSUM") as ps,
        ):
            wq_sb = cpool.tile([P, ET, H * Dh], BF, tag="wq")
            wk_sb = cpool.tile([P, ET, H * Dh], BF, tag="wk")
            wv_sb = cpool.tile([P, ET, H * Dh], BF, tag="wv")
            wo_sb = cpool.tile([P, ET, E], BF, tag="wo")
            nc.sync.dma_start(wq_sb[:], wq.rearrange("(n p) c -> p n c", p=P))
            nc.sync.dma_start(wk_sb[:], wk.rearrange("(n p) c -> p n c", p=P))
            nc.sync.dma_start(wv_sb[:], wv.rearrange("(n p) c -> p n c", p=P))
            nc.sync.dma_start(wo_sb[:], wo.rearrange("(n p) c -> p n c", p=P))
            borep_sb = cpool.tile([P, E], F32, tag="bo")
            nc.sync.dma_start(borep_sb[:], borep)
            mask_sb = cpool.tile([P, P], BF, tag="mask")
            nc.sync.dma_start(mask_sb[:], mask01)

            for b in range(BL):
                xT_sb = xpool.tile([P, ET, T], BF, tag="xT")
                nc.sync.dma_start(
                    xT_sb[:], xT[b].rearrange("(n p) c -> p n c", p=P))

                # ---- V projection: V'[tok, head, 65] (col 64 = ones) ----
                v_sb = vypool.tile([P, ET, H, Dh + 1], BF, tag="v")
                nc.vector.memset(v_sb[:, :, :, Dh], 1.0)
                for t in range(ET):
                    vp = ps.tile([P, 1024], F32, tag="ps")
                    for n2 in range(2):
                        cs = slice(512 * n2, 512 * (n2 + 1))
                        for i in range(ET):
                            nc.tensor.matmul(
                                vp[:, cs],
                                lhsT=xT_sb[:, i, 128 * t:128 * (t + 1)],
                                rhs=wv_sb[:, i, cs],
                                start=(i == 0), stop=(i == ET - 1),
                            )
                        nc.scalar.activation(
                            v_sb[:, t, 8 * n2:8 * (n2 + 1), 0:Dh],
                            vp[:, cs].rearrange("p (h d) -> p h d", d=Dh),
                            AF.Copy,
                        )

                # ---- Q^T / K^T projections (head-pair packed) ----
                qT = qkpool.tile([P, HP, T], BF, tag="q")
                kT = qkpool.tile([P, HP, T], BF, tag="k")
                for hp in range(HP):
                    qp = ps.tile([P, 1024], F32, tag="ps")
                    kp = ps.tile([P, 1024], F32, tag="ps")
                    for n2 in range(2):
                        cs = slice(512 * n2, 512 * (n2 + 1))
                        for i in range(ET):
                            nc.tensor.matmul(
                                qp[:, cs],
                                lhsT=wq_sb[:, i, 128 * hp:128 * (hp + 1)],
                                rhs=xT_sb[:, i, cs],
                                start=(i == 0), stop=(i == ET - 1),
                            )
                        for i in range(ET):
                            nc.tensor.matmul(
                                kp[:, cs],
                                lhsT=wk_sb[:, i, 128 * hp:128 * (hp + 1)],
                                rhs=xT_sb[:, i, cs],
                                start=(i == 0), stop=(i == ET - 1),
                            )
                    # PSUM->SBUF casts; attention scale folded into Q
                    nc.scalar.activation(qT[:, hp, :], qp[:], AF.Copy,
                                         scale=1.0 / float(np.sqrt(Dh)))
                    nc.scalar.activation(kT[:, hp, :], kp[:], AF.Copy)

                if dbg and b == 0:
                    nc.sync.dma_start(dbg_out["d_qT"], qT[:])
                    nc.sync.dma_start(dbg_out["d_kT"], kT[:])
                    nc.sync.dma_start(dbg_out["d_v"], v_sb[:])

                # ---- attention pass A: all heads, unnormalized ----
                yT = vypool.tile([P, HP, T], BF, tag="y")
                # per-head softmax denominators: engine APs require partition
                # base in {0,32,64,96}, so head h lands at partition
                # 32*(h%4), free slot h//4
                den = dnpool.tile([P, 4, T], F32, tag="den")
                nc.vector.memset(den[:], 1.0)

                # normalization work is queued and drip-emitted between the
                # next pair's i-steps so its DVE/GpSimd bursts never starve
                # the PE's PV chain
                pending = []

                def drain(n):
                    for _ in range(min(n, len(pending))):
                        pending.pop(0)()

                def queue_normalize(g, b=b):
                    for c in range(4):
                        def recip_chunk(g=g, c=c):
                            nc.vector.reciprocal(
                                den[:, g, 256 * c:256 * (c + 1)],
                                den[:, g, 256 * c:256 * (c + 1)])
                        pending.append(recip_chunk)
                    if dbg and b == 0 and g == 0:
                        pending.append(lambda: nc.sync.dma_start(
                            dbg_out["d_r1"], den[0:1, 0, :]))
                    for h in range(4 * g, 4 * g + 4):
                        def stage(h=h, b=b):
                            hp, po = h // 2, Dh * (h % 2)
                            pb = 32 * (h % 4)
                            r1 = spool.tile([1, T], F32, tag="r1",
                                            name=f"r1_{b}_{h}")
                            # partition_broadcast reads physical partition 0
                            # regardless of the AP base, so stage there first
                            nc.scalar.activation(
                                r1[:], den[pb:pb + 1, h // 4, :], AF.Copy)
                            rb = spool.tile([P, T], F32, tag="rb",
                                            name=f"rb_{b}_{h}")
                            nc.gpsimd.partition_broadcast(rb[:], r1[:])
                            if dbg and b == 0 and h == 1:
                                nc.sync.dma_start(dbg_out["d_rb"],
                                                  rb[0:Dh, :])
                            return rb

                        def mul(h=h, rbs=[], b=b):
                            pass
                        # stage+bcast then scale, as two drip steps sharing rb
                        holder = {}
                        pending.append(lambda h=h, holder=holder:
                                       holder.__setitem__('rb', stage(h)))
                        def mul_step(h=h, holder=holder):
                            hp, po = h // 2, Dh * (h % 2)
                            nc.vector.tensor_mul(yT[po:po + Dh, hp, :],
                                                 yT[po:po + Dh, hp, :],
                                                 holder['rb'][po:po + Dh, :])
                        pending.append(mul_step)

                for hp in range(HP):
                    # the pair's two heads interleaved so the PE always has
                    # an independent matmul while the other head's exp runs
                    ops = [ps.tile([P, 1024], F32, tag="ps",
                                   name=f"op{b}_{hp}_{s}") for s in range(2)]
                    for i in range(ET):
                        pts = []
                        for sub in (0, 1):
                            po = Dh * sub
                            sp_ = ps.tile([P, 1024], F32, tag="ps")
                            for (a0, a1) in _pieces(i):
                                nc.tensor.matmul(
                                    sp_[:, a0:a1],
                                    lhsT=kT[po:po + Dh, hp,
                                            128 * i:128 * (i + 1)],
                                    rhs=qT[po:po + Dh, hp, a0:a1],
                                    start=True, stop=True,
                                )
                            pt = ptpool.tile([P, 1024], BF, tag="pt")
                            nc.scalar.activation(pt[:, 128 * i:],
                                                 sp_[:, 128 * i:], AF.Exp)
                            ds_ = slice(128 * i, 128 * (i + 1))
                            nc.vector.tensor_mul(pt[:, ds_], pt[:, ds_],
                                                 mask_sb[:])
                            pts.append(pt)
                            if dbg and b == 0 and hp == 0 and sub == 0:
                                if i == 0:
                                    s0c = dnpool.tile([P, T], F32,
                                                      tag="dbg_s0")
                                    nc.vector.tensor_copy(s0c[:], sp_[:])
                                    nc.sync.dma_start(dbg_out["d_s0"],
                                                      s0c[:])
                                nc.sync.dma_start(dbg_out["d_pt"][i], pt[:])
                        for sub in (0, 1):
                            h = 2 * hp + sub
                            for (a0, a1) in _pieces(i):
                                nc.tensor.matmul(
                                    ops[sub][0:Dh + 1, a0:a1],
                                    lhsT=v_sb[:, i, h, :],
                                    rhs=pts[sub][:, a0:a1],
                                    start=(i == 0), stop=(i == ET - 1),
                                    skip_group_check=True,
                                )
                            if i == ET - 1:
                                # tail immediately so the PSUM slot drains
                                # while the other head still computes
                                po = Dh * sub
                                nc.scalar.activation(yT[po:po + Dh, hp, :],
                                                     ops[sub][0:Dh, :],
                                                     AF.Copy)
                                pb = 32 * (h % 4)
                                nc.vector.tensor_copy(
                                    den[pb:pb + 1, h // 4, :],
                                    ops[sub][Dh:Dh + 1, :])
                                if dbg and b == 0 and h == 0:
                                    opc = dnpool.tile([P, T], F32,
                                                      tag="dbg_op")
                                    nc.vector.tensor_copy(opc[:],
                                                          ops[sub][:])
                                    nc.sync.dma_start(dbg_out["d_op"],
                                                      opc[:])
                            drain(1)
                    if hp % 2 == 1:
                        # heads 4g..4g+3 complete: queue their reciprocal +
                        # normalize, drip-emitted during the next pair
                        queue_normalize(hp // 2)
                drain(len(pending))

                if dbg and b == 0:
                    nc.sync.dma_start(dbg_out["d_yT"], yT[:])

                # ---- output projection + bias ----
                for t in range(ET):
                    o2 = ps.tile([P, 1024], F32, tag="ps")
                    for n2 in range(2):
                        cs = slice(512 * n2, 512 * (n2 + 1))
                        for j in range(ET):
                            nc.tensor.matmul(
                                o2[:, cs],
                                lhsT=yT[:, j, 128 * t:128 * (t + 1)],
                                rhs=wo_sb[:, j, cs],
                                start=(j == 0), stop=(j == ET - 1),
                            )
                    ob = opool.tile([P, E], F32, tag="ob")
                    nc.vector.tensor_add(ob[:], o2[:], borep_sb[:])
                    nc.sync.dma_start(out[b, 128 * t:128 * (t + 1), :], ob[:])

    nc.compile()
    return nc


def _get_nc():
    if "nc" not in _CACHE:
        _CACHE["nc"] = _build()
    return _CACHE["nc"]


def _prep_in_maps(x, Wq, Wk, Wv, Wo, bo):
    bf16 = ml_dtypes.bfloat16
    # [B,T,E] -> [B,E,T] transposed activations
    xT = np.ascontiguousarray(x.transpose(0, 2, 1)).astype(bf16)
    # [H,E,Dh] -> [E, H*Dh] (heads side by side so a 128-col slice = 2 heads)
    wq_pk = np.ascontiguousarray(Wq.transpose(1, 0, 2).reshape(E, H * Dh)).astype(bf16)
    wk_pk = np.ascontiguousarray(Wk.transpose(1, 0, 2).reshape(E, H * Dh)).astype(bf16)
    wv_pk = np.ascontiguousarray(Wv.transpose(1, 0, 2).reshape(E, H * Dh)).astype(bf16)
    wo_b = np.ascontiguousarray(Wo).astype(bf16)
    borep = np.ascontiguousarray(
        np.broadcast_to(bo.astype(np.float32), (P, E)))
    ii, jj = np.mgrid[0:P, 0:P]
    mask01 = (jj >= ii).astype(bf16)  # S^T[tk,tq]: keep tq >= tk

    in_maps = []
    for c in range(NCORES):
        in_maps.append({
            "xT": xT[BL * c:BL * (c + 1)],
            "wq": wq_pk, "wk": wk_pk, "wv": wv_pk, "wo": wo_b,
            "borep": borep, "mask01": mask01,
        })
    return in_maps


def run(inputs, trace=False):
    """Returns (full_output [B,T,E] fp32, BassKernelResults)."""
    nc = _get_nc()
    in_maps = _prep_in_maps(**inputs)
    res = run_bass_kernel_spmd(nc, in_maps, core_ids=list(range(NCORES)),
                               trace=trace)
    out = np.concatenate([res.results[c]["out"] for c in range(NCORES)],
                         axis=0)
    return out, res


def kernel(x, Wq, Wk, Wv, Wo, bo):
    out, _ = run(dict(x=x, Wq=Wq, Wk=Wk, Wv=Wv, Wo=Wo, bo=bo))
    return out


# revision 33
# speedup vs baseline: 1.2498x; 1.1720x over previous
"""Causal multi-head attention (B=16, T=1024, E=1024, H=16, Dh=64) on 8 TRN2
NeuronCores.

Sharding: data-parallel over batch -- 2 batch elements per core, weights
replicated, no collectives. Host pre-transposes x and packs weights; each core
runs an identical Bass/Tile program on its shard.

Per-core dataflow (all in "transposed" orientation so no on-chip transposes
are ever needed):
  x^T [E,T] (host)   --matmul-->  Q^T,K^T [Dh,T] per head (head-pairs packed
                                  into 128 partitions; 1/sqrt(Dh) folded into
                                  the Q PSUM->SBUF copy)
                     --matmul-->  V [T,Dh] per head (+ ones column)
  S^T[tk,tq] = (K^T tile).T @ Q^T  per key-tile, causal tiles skipped
  P^T = exp(S^T) on ScalarE (scores are O(1): no max subtraction needed);
        diagonal tiles masked by 0/1 multiply
  O'^T[65,tq] += (V'|1).T @ P^T   -- row 64 accumulates the softmax denom
  Y^T = O'^T[0:64] * bcast(1/denom)
  out[t,E] = Y^T.T @ Wo + bo

Scheduling notes (hard-won on HW):
  - engine APs need partition base in {0,32,64,96}; partition_broadcast reads
    physical partition 0 regardless of the AP; tensor_tensor wants equal
    bases when both operands are SBUF.
  - HAM unthrottles the PE (1.2 -> 2.4 GHz) only on a fully-busy 3.4us
    window, so each pair's Q/K projection matmuls are interleaved into the
    previous pair's attention stream as dense filler.
  - normalization (reciprocal + broadcast + scale) is drip-emitted between
    i-steps so DVE/GpSimd bursts never starve the PE's PV chain.
"""
import numpy as np
import ml_dtypes

import concourse.bass as bass
import concourse.mybir as mybir
import concourse.tile as tile
from concourse import bacc
from concourse.bass_utils import run_bass_kernel_spmd

B, T, E = 16, 1024, 1024
H, Dh = 16, 64
NCORES = 8
BL = B // NCORES          # batches per core
P = 128                   # partitions
ET = E // P               # 8 tiles along E / token / hd dims
HP = H // 2               # 8 head-pairs
BF = mybir.dt.bfloat16
F32 = mybir.dt.float32
AF = mybir.ActivationFunctionType

_CACHE = {}


def _pieces(i):
    """Column pieces of [128*i, 1024) that do not cross the 512 PSUM-bank
    boundary."""
    if i < 4:
        return [(128 * i, 512), (512, 1024)]
    return [(128 * i, 1024)]


def _build(dbg=False):
    nc = bacc.Bacc("TRN2", target_bir_lowering=False, debug=False,
                   num_devices=NCORES)

    dbg_out = {}
    if dbg:
        for name, shape, dt in [
            ("d_qT", [P, HP, T], BF), ("d_kT", [P, HP, T], BF),
            ("d_v", [P, ET, H, Dh + 1], BF), ("d_pt", [ET, P, T], BF),
            ("d_op", [P, T], F32),
            ("d_r1", [1, T], F32), ("d_rb", [Dh, T], F32),
            ("d_yT", [P, HP, T], BF),
        ]:
            dbg_out[name] = nc.dram_tensor(name, shape, dt,
                                           kind="ExternalOutput").ap()

    xT = nc.dram_tensor("xT", [BL, E, T], BF, kind="ExternalInput").ap()
    wq = nc.dram_tensor("wq", [E, H * Dh], BF, kind="ExternalInput").ap()
    wk = nc.dram_tensor("wk", [E, H * Dh], BF, kind="ExternalInput").ap()
    wv = nc.dram_tensor("wv", [E, H * Dh], BF, kind="ExternalInput").ap()
    wo = nc.dram_tensor("wo", [H * Dh, E], BF, kind="ExternalInput").ap()
    borep = nc.dram_tensor("borep", [P, E], F32, kind="ExternalInput").ap()
    mask01 = nc.dram_tensor("mask01", [P, P], BF, kind="ExternalInput").ap()
    out = nc.dram_tensor("out", [BL, T, E], F32, kind="ExternalOutput").ap()

    with tile.TileContext(nc) as tc:
        with (
            tc.tile_pool(name="consts", bufs=1) as cpool,
            tc.tile_pool(name="xp", bufs=1) as xpool,
            tc.tile_pool(name="qk", bufs=1) as qkpool,
            tc.tile_pool(name="vy", bufs=1) as vypool,
            tc.tile_pool(name="pt", bufs=4) as ptpool,
            tc.tile_pool(name="sm", bufs=2) as spool,
            tc.tile_pool(name="dn", bufs=1) as dnpool,
            tc.tile_pool(name="ob", bufs=3) as opool,
            tc.tile_pool(name="pso", bufs=2, space="PSUM") as pso,
            tc.tile_pool(name="psc", bufs=4, space="PSUM") as psc,
        ):
            wq_sb = cpool.tile([P, ET, H * Dh], BF, tag="wq")
            wk_sb = cpool.tile([P, ET, H * Dh], BF, tag="wk")
            wv_sb = cpool.tile([P, ET, H * Dh], BF, tag="wv")
            wo_sb = cpool.tile([P, ET, E], BF, tag="wo")
            nc.sync.dma_start(wq_sb[:], wq.rearrange("(n p) c -> p n c", p=P))
            nc.sync.dma_start(wk_sb[:], wk.rearrange("(n p) c -> p n c", p=P))
            nc.sync.dma_start(wv_sb[:], wv.rearrange("(n p) c -> p n c", p=P))
            nc.sync.dma_start(wo_sb[:], wo.rearrange("(n p) c -> p n c", p=P))
            borep_sb = cpool.tile([P, E], F32, tag="bo")
            nc.sync.dma_start(borep_sb[:], borep)
            mask_sb = cpool.tile([P, P], BF, tag="mask")
            nc.sync.dma_start(mask_sb[:], mask01)

            for b in range(BL):
                xT_sb = xpool.tile([P, ET, T], BF, tag="xT",
                                   name=f"xT{b}")
                nc.sync.dma_start(
                    xT_sb[:], xT[b].rearrange("(n p) c -> p n c", p=P))

                # ---- V projection: V'[tok, head, 65] (col 64 = ones) ----
                v_sb = vypool.tile([P, ET, H, Dh + 1], BF, tag="v",
                                   name=f"v{b}")
                nc.vector.memset(v_sb[:, :, :, Dh], 1.0)
                for t in range(ET):
                    for n2 in range(2):
                        cs = slice(512 * n2, 512 * (n2 + 1))
                        vp = psc.tile([P, 512], F32, tag="pc",
                                      name=f"vp{b}_{t}_{n2}")
                        for i in range(ET):
                            nc.tensor.matmul(
                                vp[:],
                                lhsT=xT_sb[:, i, 128 * t:128 * (t + 1)],
                                rhs=wv_sb[:, i, cs],
                                start=(i == 0), stop=(i == ET - 1),
                            )
                        nc.scalar.activation(
                            v_sb[:, t, 8 * n2:8 * (n2 + 1), 0:Dh],
                            vp[:].rearrange("p (h d) -> p h d", d=Dh),
                            AF.Copy,
                        )

                # ---- Q^T / K^T projections, emitted as closures so pair
                # pp's projection interleaves into pair pp-1's attention ----
                qT = qkpool.tile([P, HP, T], BF, tag="q", name=f"q{b}")
                kT = qkpool.tile([P, HP, T], BF, tag="k", name=f"k{b}")

                def proj_subblocks(pp, b=b, qT=qT, kT=kT, xT_sb=xT_sb):
                    blocks = []
                    for (lbl, w_sb, dst, scale) in (("q", wq_sb, qT, 0.125),
                                                    ("k", wk_sb, kT, 1.0)):
                        for n2 in range(2):
                            def blk(lbl=lbl, w_sb=w_sb, dst=dst, scale=scale,
                                    n2=n2, pp=pp, b=b):
                                cs = slice(512 * n2, 512 * (n2 + 1))
                                pj = psc.tile(
                                    [P, 512], F32, tag="pc",
                                    name=f"pj{b}_{pp}_{n2}_{lbl}")
                                for i in range(ET):
                                    nc.tensor.matmul(
                                        pj[:],
                                        lhsT=w_sb[:, i,
                                                  128 * pp:128 * (pp + 1)],
                                        rhs=xT_sb[:, i, cs],
                                        start=(i == 0), stop=(i == ET - 1),
                                    )
                                nc.scalar.activation(dst[:, pp, cs], pj[:],
                                                     AF.Copy, scale=scale)
                            blocks.append(blk)
                    return blocks

                for blk in proj_subblocks(0):
                    blk()

                # ---- attention: pairs of heads, drip-scheduled extras ----
                yT = vypool.tile([P, HP, T], BF, tag="y", name=f"y{b}")
                den = dnpool.tile([P, 4, T], F32, tag="den")
                nc.vector.memset(den[:], 1.0)

                pending = []

                def drain(n):
                    for _ in range(min(n, len(pending))):
                        pending.pop(0)()

                def queue_normalize(g, b=b, yT=yT, den=den):
                    for c in range(4):
                        def recip_chunk(g=g, c=c):
                            nc.vector.reciprocal(
                                den[:, g, 256 * c:256 * (c + 1)],
                                den[:, g, 256 * c:256 * (c + 1)])
                        pending.append(recip_chunk)
                    if dbg and b == 0 and g == 0:
                        pending.append(lambda: nc.sync.dma_start(
                            dbg_out["d_r1"], den[0:1, 0, :]))
                    for h in range(4 * g, 4 * g + 4):
                        holder = {}

                        def stage(h=h, b=b, holder=None):
                            hp, po = h // 2, Dh * (h % 2)
                            pb = 32 * (h % 4)
                            r1 = spool.tile([1, T], F32, tag="r1",
                                            name=f"r1_{b}_{h}")
                            nc.scalar.activation(
                                r1[:], den[pb:pb + 1, h // 4, :], AF.Copy)
                            rb = spool.tile([P, T], F32, tag="rb",
                                            name=f"rb_{b}_{h}")
                            nc.gpsimd.partition_broadcast(rb[:], r1[:])
                            if dbg and b == 0 and h == 1:
                                nc.sync.dma_start(dbg_out["d_rb"],
                                                  rb[0:Dh, :])
                            holder['rb'] = rb

                        def mul_step(h=h, holder=holder):
                            hp, po = h // 2, Dh * (h % 2)
                            nc.vector.tensor_mul(
                                yT[po:po + Dh, hp, :],
                                yT[po:po + Dh, hp, :],
                                holder['rb'][po:po + Dh, :])

                        pending.append(
                            lambda h=h, holder=holder: stage(h, b, holder))
                        pending.append(mul_step)

                for hp in range(HP):
                    if hp + 1 < HP:
                        # front of the queue: pair hp+1's projection must
                        # finish within this pair's attention
                        pending[0:0] = proj_subblocks(hp + 1)
                    ops = [pso.tile([P, 1024], F32, tag="op",
                                    name=f"op{b}_{hp}_{s}") for s in range(2)]
                    for i in range(ET):
                        pts = []
                        for sub in (0, 1):
                            po = Dh * sub
                            pt = ptpool.tile([P, 1024], BF, tag="pt",
                                             name=f"pt{b}_{hp}_{i}_{sub}")
                            for (a0, a1) in _pieces(i):
                                sp_ = psc.tile([P, 512], F32, tag="pc",
                                               name=f"sp{b}_{hp}_{i}_{sub}_{a0}")
                                w = a1 - a0
                                nc.tensor.matmul(
                                    sp_[:, 0:w],
                                    lhsT=kT[po:po + Dh, hp,
                                            128 * i:128 * (i + 1)],
                                    rhs=qT[po:po + Dh, hp, a0:a1],
                                    start=True, stop=True,
                                )
                                nc.scalar.activation(pt[:, a0:a1],
                                                     sp_[:, 0:w], AF.Exp)
                            ds_ = slice(128 * i, 128 * (i + 1))
                            nc.vector.tensor_mul(pt[:, ds_], pt[:, ds_],
                                                 mask_sb[:])
                            pts.append(pt)
                            if dbg and b == 0 and hp == 0 and sub == 0:
                                nc.sync.dma_start(dbg_out["d_pt"][i], pt[:])
                        for sub in (0, 1):
                            h = 2 * hp + sub
                            for (a0, a1) in _pieces(i):
                                nc.tensor.matmul(
                                    ops[sub][0:Dh + 1, a0:a1],
                                    lhsT=v_sb[:, i, h, :],
                                    rhs=pts[sub][:, a0:a1],
                                    start=(i == 0), stop=(i == ET - 1),
                                    skip_group_check=True,
                                )
                            if i == ET - 1:
                                po = Dh * sub
                                nc.scalar.activation(yT[po:po + Dh, hp, :],
                                                     ops[sub][0:Dh, :],
                                                     AF.Copy)
                                pb = 32 * (h % 4)
                                nc.vector.tensor_copy(
                                    den[pb:pb + 1, h // 4, :],
                                    ops[sub][Dh:Dh + 1, :])
                                if dbg and b == 0 and h == 0:
                                    opc = dnpool.tile([P, T], F32,
                                                      tag="dbg_op")
                                    nc.vector.tensor_copy(opc[:],
                                                          ops[sub][:])
                                    nc.sync.dma_start(dbg_out["d_op"],
                                                      opc[:])
                            drain(1)
                    if hp % 2 == 1:
                        queue_normalize(hp // 2)

                # emission order IS dependency order under Tile's tracer:
                # all normalize muls must be emitted before out-proj reads yT
                drain(len(pending))
                if dbg and b == 0:
                    nc.sync.dma_start(dbg_out["d_qT"], qT[:])
                    nc.sync.dma_start(dbg_out["d_kT"], kT[:])
                    nc.sync.dma_start(dbg_out["d_v"], v_sb[:])
                    nc.sync.dma_start(dbg_out["d_yT"], yT[:])

                # ---- output projection + bias ----
                for t in range(ET):
                    for n2 in range(2):
                        cs = slice(512 * n2, 512 * (n2 + 1))
                        o2 = psc.tile([P, 512], F32, tag="pc",
                                      name=f"o2_{b}_{t}_{n2}")
                        for j in range(ET):
                            nc.tensor.matmul(
                                o2[:],
                                lhsT=yT[:, j, 128 * t:128 * (t + 1)],
                                rhs=wo_sb[:, j, cs],
                                start=(j == 0), stop=(j == ET - 1),
                            )
                        ob = opool.tile([P, 512], F32, tag="ob",
                                        name=f"ob{b}_{t}_{n2}")
                        nc.vector.tensor_add(ob[:], o2[:], borep_sb[:, cs])
                        nc.sync.dma_start(out[b, 128 * t:128 * (t + 1), cs],
                                          ob[:])
                drain(len(pending))

    nc.compile()
    return nc


def _get_nc():
    if "nc" not in _CACHE:
        _CACHE["nc"] = _build()
    return _CACHE["nc"]


def _prep_in_maps(x, Wq, Wk, Wv, Wo, bo):
    bf16 = ml_dtypes.bfloat16
    # [B,T,E] -> [B,E,T] transposed activations
    xT = np.ascontiguousarray(np.asarray(x).transpose(0, 2, 1)).astype(bf16)
    # [H,E,Dh] -> [E, H*Dh] (heads side by side so a 128-col slice = 2 heads)
    wq_pk = np.ascontiguousarray(
        np.asarray(Wq).transpose(1, 0, 2).reshape(E, H * Dh)).astype(bf16)
    wk_pk = np.ascontiguousarray(
        np.asarray(Wk).transpose(1, 0, 2).reshape(E, H * Dh)).astype(bf16)
    wv_pk = np.ascontiguousarray(
        np.asarray(Wv).transpose(1, 0, 2).reshape(E, H * Dh)).astype(bf16)
    wo_b = np.ascontiguousarray(np.asarray(Wo)).astype(bf16)
    borep = np.ascontiguousarray(
        np.broadcast_to(np.asarray(bo, np.float32), (P, E)))
    ii, jj = np.mgrid[0:P, 0:P]
    mask01 = (jj >= ii).astype(bf16)  # S^T[tk,tq]: keep tq >= tk

    in_maps = []
    for c in range(NCORES):
        in_maps.append({
            "xT": xT[BL * c:BL * (c + 1)],
            "wq": wq_pk, "wk": wk_pk, "wv": wv_pk, "wo": wo_b,
            "borep": borep, "mask01": mask01,
        })
    return in_maps


def run(inputs, trace=False):
    """Returns (full_output [B,T,E] fp32, BassKernelResults)."""
    nc = _get_nc()
    in_maps = _prep_in_maps(**inputs)
    res = run_bass_kernel_spmd(nc, in_maps, core_ids=list(range(NCORES)),
                               trace=trace)
    out = np.concatenate([res.results[c]["out"] for c in range(NCORES)],
                         axis=0)
    return out, res


def kernel(x, Wq, Wk, Wv, Wo, bo):
    out, _ = run(dict(x=x, Wq=Wq, Wk=Wk, Wv=Wv, Wo=Wo, bo=bo))
    return out


# revision 38
# speedup vs baseline: 1.3672x; 1.0939x over previous
"""Causal multi-head attention (B=16, T=1024, E=1024, H=16, Dh=64) on 8 TRN2
NeuronCores.

Sharding: data-parallel over batch -- 2 batch elements per core, weights
replicated, no collectives. Host pre-transposes x and packs weights; each core
runs an identical Bass/Tile program on its shard.

Per-core dataflow (all in "transposed" orientation so no on-chip transposes
are ever needed):
  x^T [E,T] (host)   --matmul-->  Q^T,K^T [Dh,T] per head (head-pairs packed
                                  into 128 partitions; 1/sqrt(Dh) folded into
                                  the Q PSUM->SBUF copy)
                     --matmul-->  V [T,Dh] per head (+ ones column)
  S^T[tk,tq] = (K^T tile).T @ Q^T  per key-tile, causal tiles skipped
  P^T = exp(S^T) on ScalarE (scores are O(1): no max subtraction needed);
        diagonal tiles masked by 0/1 multiply
  O'^T[65,tq] += (V'|1).T @ P^T   -- row 64 accumulates the softmax denom
  Y^T = O'^T[0:64] * bcast(1/denom)
  out[t,E] = Y^T.T @ Wo + bo

Scheduling notes (hard-won on HW):
  - engine APs need partition base in {0,32,64,96}; partition_broadcast reads
    physical partition 0 regardless of the AP; tensor_tensor wants equal
    bases when both operands are SBUF.
  - HAM unthrottles the PE (1.2 -> 2.4 GHz) only on a fully-busy 3.4us
    window, so each pair's Q/K projection matmuls are interleaved into the
    previous pair's attention stream as dense filler.
  - normalization (reciprocal + broadcast + scale) is drip-emitted between
    i-steps so DVE/GpSimd bursts never starve the PE's PV chain.
"""
import numpy as np
import ml_dtypes

import concourse.bass as bass
import concourse.mybir as mybir
import concourse.tile as tile
from concourse import bacc
from concourse.bass_utils import run_bass_kernel_spmd

B, T, E = 16, 1024, 1024
H, Dh = 16, 64
NCORES = 8
BL = B // NCORES          # batches per core
P = 128                   # partitions
ET = E // P               # 8 tiles along E / token / hd dims
HP = H // 2               # 8 head-pairs
BF = mybir.dt.bfloat16
F32 = mybir.dt.float32
AF = mybir.ActivationFunctionType

_CACHE = {}


def _pieces(i):
    """Column pieces of [128*i, 1024) that do not cross the 512 PSUM-bank
    boundary."""
    if i < 4:
        return [(128 * i, 512), (512, 1024)]
    return [(128 * i, 1024)]


def _build(dbg=False):
    nc = bacc.Bacc("TRN2", target_bir_lowering=False, debug=False,
                   num_devices=NCORES)

    dbg_out = {}
    if dbg:
        for name, shape, dt in [
            ("d_qT", [P, HP, T], BF), ("d_kT", [P, HP, T], BF),
            ("d_v", [P, ET, H, Dh + 1], BF), ("d_pt", [ET, P, T], BF),
            ("d_op", [P, T], F32),
            ("d_r1", [1, T], F32), ("d_rb", [Dh, T], F32),
            ("d_yT", [P, HP, T], BF),
        ]:
            dbg_out[name] = nc.dram_tensor(name, shape, dt,
                                           kind="ExternalOutput").ap()

    xT = nc.dram_tensor("xT", [BL, E, T], BF, kind="ExternalInput").ap()
    wq = nc.dram_tensor("wq", [E, H * Dh], BF, kind="ExternalInput").ap()
    wk = nc.dram_tensor("wk", [E, H * Dh], BF, kind="ExternalInput").ap()
    wv = nc.dram_tensor("wv", [E, H * Dh], BF, kind="ExternalInput").ap()
    wo = nc.dram_tensor("wo", [H * Dh, E], BF, kind="ExternalInput").ap()
    borep = nc.dram_tensor("borep", [P, E], F32, kind="ExternalInput").ap()
    mask01 = nc.dram_tensor("mask01", [P, P], BF, kind="ExternalInput").ap()
    out = nc.dram_tensor("out", [BL, T, E], F32, kind="ExternalOutput").ap()

    with tile.TileContext(nc) as tc:
        with (
            tc.tile_pool(name="consts", bufs=1) as cpool,
            tc.tile_pool(name="xp", bufs=1) as xpool,
            tc.tile_pool(name="qk", bufs=1) as qkpool,
            tc.tile_pool(name="vp2", bufs=2) as vpool,
            tc.tile_pool(name="vy", bufs=1) as vypool,
            tc.tile_pool(name="pt", bufs=4) as ptpool,
            tc.tile_pool(name="sm", bufs=2) as spool,
            tc.tile_pool(name="dn", bufs=1) as dnpool,
            tc.tile_pool(name="ob", bufs=3) as opool,
            tc.tile_pool(name="pso", bufs=2, space="PSUM") as pso,
            tc.tile_pool(name="psc", bufs=4, space="PSUM") as psc,
        ):
            wq_sb = cpool.tile([P, ET, H * Dh], BF, tag="wq")
            wk_sb = cpool.tile([P, ET, H * Dh], BF, tag="wk")
            wv_sb = cpool.tile([P, ET, H * Dh], BF, tag="wv")
            wo_sb = cpool.tile([P, ET, E], BF, tag="wo")
            nc.sync.dma_start(wq_sb[:], wq.rearrange("(n p) c -> p n c", p=P))
            nc.sync.dma_start(wk_sb[:], wk.rearrange("(n p) c -> p n c", p=P))
            nc.sync.dma_start(wv_sb[:], wv.rearrange("(n p) c -> p n c", p=P))
            nc.sync.dma_start(wo_sb[:], wo.rearrange("(n p) c -> p n c", p=P))
            borep_sb = cpool.tile([P, E], F32, tag="bo")
            nc.sync.dma_start(borep_sb[:], borep)
            mask_sb = cpool.tile([P, P], BF, tag="mask")
            nc.sync.dma_start(mask_sb[:], mask01)

            xT_tiles = {}
            v_tiles = {}

            def load_blocks(b):
                """xT load + V-projection for batch b as dense PE filler
                blocks (also usable as pending entries during the previous
                batch's last attention pairs)."""
                def ld(b=b):
                    xT_tiles[b] = xpool.tile([P, ET, T], BF, tag="xT",
                                             name=f"xT{b}")
                    nc.sync.dma_start(
                        xT_tiles[b][:],
                        xT[b].rearrange("(n p) c -> p n c", p=P))
                    v_tiles[b] = vpool.tile([P, ET, H, Dh + 1], BF, tag="v",
                                            name=f"v{b}")
                    nc.vector.memset(v_tiles[b][:, :, :, Dh], 1.0)
                blocks = [ld]
                for t in range(ET):
                    for n2 in range(2):
                        def vblk(t=t, n2=n2, b=b):
                            cs = slice(512 * n2, 512 * (n2 + 1))
                            vp = psc.tile([P, 512], F32, tag="pc",
                                          name=f"vp{b}_{t}_{n2}")
                            for i in range(ET):
                                nc.tensor.matmul(
                                    vp[:],
                                    lhsT=xT_tiles[b][:, i,
                                                     128 * t:128 * (t + 1)],
                                    rhs=wv_sb[:, i, cs],
                                    start=(i == 0), stop=(i == ET - 1),
                                )
                            nc.scalar.activation(
                                v_tiles[b][:, t, 8 * n2:8 * (n2 + 1), 0:Dh],
                                vp[:].rearrange("p (h d) -> p h d", d=Dh),
                                AF.Copy,
                            )
                        blocks.append(vblk)
                return blocks

            for blk in load_blocks(0):
                blk()

            for b in range(BL):
                xT_sb = xT_tiles[b]
                v_sb = v_tiles[b]

                # ---- Q^T / K^T projections, emitted as closures so pair
                # pp's projection interleaves into pair pp-1's attention ----
                qT = qkpool.tile([P, HP, T], BF, tag="q", name=f"q{b}")
                kT = qkpool.tile([P, HP, T], BF, tag="k", name=f"k{b}")

                def proj_subblocks(pp, b=b, qT=qT, kT=kT, xT_sb=xT_sb):
                    blocks = []
                    for (lbl, w_sb, dst, scale) in (("q", wq_sb, qT, 0.125),
                                                    ("k", wk_sb, kT, 1.0)):
                        for n2 in range(2):
                            def blk(lbl=lbl, w_sb=w_sb, dst=dst, scale=scale,
                                    n2=n2, pp=pp, b=b):
                                cs = slice(512 * n2, 512 * (n2 + 1))
                                pj = psc.tile(
                                    [P, 512], F32, tag="pc",
                                    name=f"pj{b}_{pp}_{n2}_{lbl}")
                                for i in range(ET):
                                    nc.tensor.matmul(
                                        pj[:],
                                        lhsT=w_sb[:, i,
                                                  128 * pp:128 * (pp + 1)],
                                        rhs=xT_sb[:, i, cs],
                                        start=(i == 0), stop=(i == ET - 1),
                                    )
                                nc.scalar.activation(dst[:, pp, cs], pj[:],
                                                     AF.Copy, scale=scale)
                            blocks.append(blk)
                    return blocks

                for blk in proj_subblocks(0):
                    blk()

                # ---- attention: pairs of heads, drip-scheduled extras ----
                yT = vypool.tile([P, HP, T], BF, tag="y", name=f"y{b}")
                den = dnpool.tile([P, 4, T], F32, tag="den")
                nc.vector.memset(den[:], 1.0)

                pending = []

                def drain(n):
                    for _ in range(min(n, len(pending))):
                        pending.pop(0)()

                def queue_normalize(g, b=b, yT=yT, den=den):
                    for c in range(8):
                        def recip_chunk(g=g, c=c):
                            nc.vector.reciprocal(
                                den[:, g, 128 * c:128 * (c + 1)],
                                den[:, g, 128 * c:128 * (c + 1)])
                        pending.append(recip_chunk)
                    if dbg and b == 0 and g == 0:
                        pending.append(lambda: nc.sync.dma_start(
                            dbg_out["d_r1"], den[0:1, 0, :]))
                    for h in range(4 * g, 4 * g + 4):
                        holder = {}

                        def stage(h=h, b=b, holder=None):
                            hp, po = h // 2, Dh * (h % 2)
                            pb = 32 * (h % 4)
                            r1 = spool.tile([1, T], BF, tag="r1",
                                            name=f"r1_{b}_{h}")
                            nc.scalar.activation(
                                r1[:], den[pb:pb + 1, h // 4, :], AF.Copy)
                            rb = spool.tile([P, T], BF, tag="rb",
                                            name=f"rb_{b}_{h}")
                            nc.gpsimd.partition_broadcast(rb[:], r1[:])
                            if dbg and b == 0 and h == 1:
                                nc.sync.dma_start(dbg_out["d_rb"],
                                                  rb[0:Dh, :])
                            holder['rb'] = rb

                        def mul_step(h=h, holder=holder):
                            hp, po = h // 2, Dh * (h % 2)
                            nc.vector.tensor_mul(
                                yT[po:po + Dh, hp, :],
                                yT[po:po + Dh, hp, :],
                                holder['rb'][po:po + Dh, :])

                        pending.append(
                            lambda h=h, holder=holder: stage(h, b, holder))
                        pending.append(mul_step)

                for hp in range(HP):
                    if hp + 1 < HP:
                        # front of the queue: pair hp+1's projection must
                        # finish within this pair's attention
                        pending[0:0] = proj_subblocks(hp + 1)
                    if hp == 6 and b + 1 < BL:
                        # next batch's x load + V projection: dense PE
                        # filler for the last two pairs (which have no
                        # projection blocks of their own)
                        pending.extend(load_blocks(b + 1))
                    ops = [pso.tile([P, 1024], F32, tag="op",
                                    name=f"op{b}_{hp}_{s}") for s in range(2)]
                    for i in range(ET):
                        pts = []
                        for sub in (0, 1):
                            po = Dh * sub
                            pt = ptpool.tile([P, 1024], BF, tag="pt",
                                             name=f"pt{b}_{hp}_{i}_{sub}")
                            for (a0, a1) in _pieces(i):
                                sp_ = psc.tile([P, 512], F32, tag="pc",
                                               name=f"sp{b}_{hp}_{i}_{sub}_{a0}")
                                w = a1 - a0
                                nc.tensor.matmul(
                                    sp_[:, 0:w],
                                    lhsT=kT[po:po + Dh, hp,
                                            128 * i:128 * (i + 1)],
                                    rhs=qT[po:po + Dh, hp, a0:a1],
                                    start=True, stop=True,
                                )
                                nc.scalar.activation(pt[:, a0:a1],
                                                     sp_[:, 0:w], AF.Exp)
                            ds_ = slice(128 * i, 128 * (i + 1))
                            nc.vector.tensor_mul(pt[:, ds_], pt[:, ds_],
                                                 mask_sb[:])
                            pts.append(pt)
                            if dbg and b == 0 and hp == 0 and sub == 0:
                                nc.sync.dma_start(dbg_out["d_pt"][i], pt[:])
                        for sub in (0, 1):
                            h = 2 * hp + sub
                            for (a0, a1) in _pieces(i):
                                nc.tensor.matmul(
                                    ops[sub][0:Dh + 1, a0:a1],
                                    lhsT=v_sb[:, i, h, :],
                                    rhs=pts[sub][:, a0:a1],
                                    start=(i == 0), stop=(i == ET - 1),
                                    skip_group_check=True,
                                )
                            if i == ET - 1:
                                po = Dh * sub
                                nc.scalar.activation(yT[po:po + Dh, hp, :],
                                                     ops[sub][0:Dh, :],
                                                     AF.Copy)
                                pb = 32 * (h % 4)
                                nc.vector.tensor_copy(
                                    den[pb:pb + 1, h // 4, :],
                                    ops[sub][Dh:Dh + 1, :])
                                if dbg and b == 0 and h == 0:
                                    opc = dnpool.tile([P, T], F32,
                                                      tag="dbg_op")
                                    nc.vector.tensor_copy(opc[:],
                                                          ops[sub][:])
                                    nc.sync.dma_start(dbg_out["d_op"],
                                                      opc[:])
                            drain(1)
                    if hp % 2 == 1:
                        queue_normalize(hp // 2)

                # emission order IS dependency order under Tile's tracer:
                # all normalize muls must be emitted before out-proj reads yT
                drain(len(pending))
                if dbg and b == 0:
                    nc.sync.dma_start(dbg_out["d_qT"], qT[:])
                    nc.sync.dma_start(dbg_out["d_kT"], kT[:])
                    nc.sync.dma_start(dbg_out["d_v"], v_sb[:])
                    nc.sync.dma_start(dbg_out["d_yT"], yT[:])

                # ---- output projection + bias ----
                for t in range(ET):
                    for n2 in range(2):
                        cs = slice(512 * n2, 512 * (n2 + 1))
                        o2 = psc.tile([P, 512], F32, tag="pc",
                                      name=f"o2_{b}_{t}_{n2}")
                        for j in range(ET):
                            nc.tensor.matmul(
                                o2[:],
                                lhsT=yT[:, j, 128 * t:128 * (t + 1)],
                                rhs=wo_sb[:, j, cs],
                                start=(j == 0), stop=(j == ET - 1),
                            )
                        ob = opool.tile([P, 512], F32, tag="ob",
                                        name=f"ob{b}_{t}_{n2}")
                        nc.vector.tensor_add(ob[:], o2[:], borep_sb[:, cs])
                        nc.sync.dma_start(out[b, 128 * t:128 * (t + 1), cs],
                                          ob[:])
                drain(len(pending))

    nc.compile()
    return nc


def _get_nc():
    if "nc" not in _CACHE:
        _CACHE["nc"] = _build()
    return _CACHE["nc"]


def _prep_in_maps(x, Wq, Wk, Wv, Wo, bo):
    bf16 = ml_dtypes.bfloat16
    # [B,T,E] -> [B,E,T] transposed activations
    xT = np.ascontiguousarray(np.asarray(x).transpose(0, 2, 1)).astype(bf16)
    # [H,E,Dh] -> [E, H*Dh] (heads side by side so a 128-col slice = 2 heads)
    wq_pk = np.ascontiguousarray(
        np.asarray(Wq).transpose(1, 0, 2).reshape(E, H * Dh)).astype(bf16)
    wk_pk = np.ascontiguousarray(
        np.asarray(Wk).transpose(1, 0, 2).reshape(E, H * Dh)).astype(bf16)
    wv_pk = np.ascontiguousarray(
        np.asarray(Wv).transpose(1, 0, 2).reshape(E, H * Dh)).astype(bf16)
    wo_b = np.ascontiguousarray(np.asarray(Wo)).astype(bf16)
    borep = np.ascontiguousarray(
        np.broadcast_to(np.asarray(bo, np.float32), (P, E)))
    ii, jj = np.mgrid[0:P, 0:P]
    mask01 = (jj >= ii).astype(bf16)  # S^T[tk,tq]: keep tq >= tk

    in_maps = []
    for c in range(NCORES):
        in_maps.append({
            "xT": xT[BL * c:BL * (c + 1)],
            "wq": wq_pk, "wk": wk_pk, "wv": wv_pk, "wo": wo_b,
            "borep": borep, "mask01": mask01,
        })
    return in_maps


def run(inputs, trace=False):
    """Returns (full_output [B,T,E] fp32, BassKernelResults)."""
    nc = _get_nc()
    in_maps = _prep_in_maps(**inputs)
    res = run_bass_kernel_spmd(nc, in_maps, core_ids=list(range(NCORES)),
                               trace=trace)
    out = np.concatenate([res.results[c]["out"] for c in range(NCORES)],
                         axis=0)
    return out, res


def kernel(x, Wq, Wk, Wv, Wo, bo):
    out, _ = run(dict(x=x, Wq=Wq, Wk=Wk, Wv=Wv, Wo=Wo, bo=bo))
    return out
